# revision 15
# baseline (speedup 1.0000x reference)
"""Causal attention head kernel for Trainium2, 8 NeuronCores.

Problem: B=4, S=4096, D_IN=512, D_OUT=64, f32, causal, scale=1/sqrt(S).

Sharding: core c -> (batch b = c//2, k-shard hk = c%2). Each core handles ALL
queries of its batch but only the k-tiles (of 128 rows) with tile_index % 2 == hk.
The instruction stream is identical across cores (SPMD); causality differences
between the two k-shards are folded into a small per-core mask input (cmask2)
and the per-core gather of X_k/X_v rows. Host pre-transposes X (layout prep)
so the device can load d-on-partitions tiles with large contiguous DMAs.

Per-core device pipeline (f32 datapath, float32r matmuls: 1 cyc/row at N>=256):
  1. DMA X^T d-blocks: xq [4x(128,4096)], xk/xv [4x(128,2048)] f32.
  2. Projections: QT[64,4096], KT[64,2048] (head dim on partitions) and
     V_aug[128k, 65] (natural layout + ones column -> softmax denominator).
  3. Per q-chunk (512) loop over k-tile PAIRS: S^T = KT_tile.T @ QT_chunk into
     a [128,1024] PSUM pair, one Exp activation (PSUM->SBUF f32; no max
     subtraction -- scores are O(1) by construction), diagonal-pair mask via
     elementwise mul, PV accumulation with ones-augmented V: psum[65,512].
  4. Output partial numerator|denominator PVT [65,4096] f32 per core.

Host combines: out[b] = ((PVT[2b] + PVT[2b+1])[0:64] / [64]).T
"""

import numpy as np

import concourse.bass as bass
import concourse.bacc as bacc
import concourse.mybir as mybir
from concourse import tile
from concourse.bass_utils import run_bass_kernel_spmd

F32 = mybir.dt.float32
F32R = mybir.dt.float32r

B, S, D_IN, D_OUT = 4, 4096, 512, 64
SK = S // 2          # per-core k rows (interleaved 128-tiles)
N_KT = SK // 128     # 16 local k-tiles
N_QC = S // 512      # 8 q-chunks of 512
N_CORES = 8

_CACHE = {}


def mm(nc, out, lhsT, rhs, start, stop):
    nc.tensor.matmul(out, lhsT, rhs, start=start, stop=stop)


def build_nc():
    nc = bacc.Bacc(trn_type="TRN2", target_bir_lowering=False, debug=False)

    xqt_d = nc.dram_tensor("xqt", [D_IN, S], F32R, kind="ExternalInput").ap()
    xkt_d = nc.dram_tensor("xkt", [D_IN, SK], F32R, kind="ExternalInput").ap()
    xvt_d = nc.dram_tensor("xvt", [D_IN, SK], F32R, kind="ExternalInput").ap()
    wq = nc.dram_tensor("wq", [128, 4, D_OUT], F32R, kind="ExternalInput").ap()
    wk = nc.dram_tensor("wk", [128, 4, D_OUT], F32R, kind="ExternalInput").ap()
    wv = nc.dram_tensor("wv", [128, 4, D_OUT], F32R, kind="ExternalInput").ap()
    cm = nc.dram_tensor("cmask2", [128, 1024], F32R, kind="ExternalInput").ap()
    ones_d = nc.dram_tensor("ones16", [128, N_KT], F32R, kind="ExternalInput").ap()
    pvt = nc.dram_tensor("pvt", [D_OUT + 1, S], F32, kind="ExternalOutput").ap()

    with tile.TileContext(nc) as tc:
        with (
            tc.tile_pool(name="persist", bufs=1) as pp,
            tc.tile_pool(name="et", bufs=3) as etp,
            tc.tile_pool(name="ostage", bufs=2) as osp,
            tc.tile_pool(name="ps_s", bufs=2, space="PSUM") as ps_s,
            tc.tile_pool(name="ps_pv", bufs=2, space="PSUM") as ps_pv,
            tc.tile_pool(name="ps_pr", bufs=2, space="PSUM") as ps_pr,
        ):
            # ---- persistent SBUF tiles ----
            xqT = [pp.tile([128, S], F32R, tag=f"xqT{i}", name=f"xqT{i}")
                   for i in range(4)]
            xkT = [pp.tile([128, SK], F32R, tag=f"xkT{i}", name=f"xkT{i}")
                   for i in range(4)]
            xvT = [pp.tile([128, SK], F32R, tag=f"xvT{i}", name=f"xvT{i}")
                   for i in range(4)]
            qt = pp.tile([64, S], F32R, tag="qt", name="qt")
            kt = pp.tile([64, SK], F32R, tag="kt", name="kt")
            vaug = pp.tile([128, N_KT, D_OUT + 1], F32R, tag="vaug", name="vaug")
            cmask = pp.tile([128, 1024], F32R, tag="cmask", name="cmask")
            w_sb = {}
            for nm, src in (("wq", wq), ("wk", wk), ("wv", wv)):
                w_sb[nm] = pp.tile([128, 4, D_OUT], F32R, tag=nm, name=f"{nm}_sb")
                nc.sync.dma_start(out=w_sb[nm][:], in_=src[:])
            nc.sync.dma_start(out=cmask[:], in_=cm[:])
            # ones column for the softmax denominator
            nc.sync.dma_start(out=vaug[:, :, D_OUT], in_=ones_d[:])

            # ---- software-pipelined loads -> projections -> attention ----
            # Column-chunked loads (512 cols = 256 KiB per DMA) so q-chunk 0's
            # dependencies clear early and later loads overlap attention.
            def load_cols(dst_tiles, src_ap, c):
                sl = slice(c * 512, (c + 1) * 512)
                for db in range(4):
                    nc.sync.dma_start(
                        out=dst_tiles[db][:, sl],
                        in_=src_ap[db * 128:(db + 1) * 128, sl])

            def proj_chunk(dst, xT, w, c, nm):
                sl = slice(c * 512, (c + 1) * 512)
                ps = ps_pr.tile([128, 512], F32, tag="ps_pr", name=f"pp_{nm}{c}")
                for dt in range(4):
                    mm(nc, ps[0:64, :], w[:, dt, :], xT[dt][:, sl],
                       start=(dt == 0), stop=(dt == 3))
                nc.vector.tensor_copy(dst[:, sl], ps[0:64, :])

            def v_chunk(c):
                # V natural for s-tiles 4c..4c+3: [128s,64] per tile
                for t in range(4 * c, 4 * c + 4):
                    ps = ps_pr.tile([128, 512], F32, tag="ps_pr", name=f"pv_{t}")
                    for dt in range(4):
                        mm(nc, ps[:, 0:D_OUT],
                           xvT[dt][:, t * 128:(t + 1) * 128],
                           w_sb["wv"][:, dt, :],
                           start=(dt == 0), stop=(dt == 3))
                    nc.vector.tensor_copy(vaug[:, t, 0:D_OUT], ps[:, 0:D_OUT])

            # SBUF accumulators for the k-chunk-outer PV partial sums
            acc = [pp.tile([65, 512], F32, tag=f"acc{j}", name=f"acc{j}")
                   for j in range(N_QC)]

            def kv_stage(kc):
                load_cols(xkT, xkt_d, kc)
                load_cols(xvT, xvt_d, kc)
                proj_chunk(kt, xkT, w_sb["wk"], kc, "k")
                v_chunk(kc)

            def q_stage(j):
                load_cols(xqT, xqt_d, j)
                proj_chunk(qt, xqT, w_sb["wq"], j, "q")

            def attn_block(j, kc):
                # pairs i of q-chunk j whose k-tiles (2i, 2i+1) lie in k-chunk
                # kc; i == j is the diagonal (masked) pair.
                iis = [i for i in (2 * kc, 2 * kc + 1) if i <= j]
                if not iis:
                    return
                qs = qt[:, j * 512:(j + 1) * 512]
                pv = ps_pv.tile([65, 512], F32, tag="ps_pv", name=f"pvp{j}_{kc}")
                for n, i in enumerate(iis):
                    ps = ps_s.tile([128, 1024], F32, tag="ps_s", name=f"st{j}_{i}")
                    for h in range(2):
                        t = 2 * i + h
                        mm(nc, ps[:, h * 512:(h + 1) * 512],
                           kt[:, t * 128:(t + 1) * 128], qs,
                           start=True, stop=True)
                    et = etp.tile([128, 1024], F32R, tag="et", name=f"et{j}_{i}")
                    nc.scalar.activation(
                        et[:], ps[:], mybir.ActivationFunctionType.Exp)
                    if i == j:  # diagonal pair: causal mask
                        nc.vector.tensor_mul(et[:], et[:], cmask[:])
                    for h in range(2):
                        t = 2 * i + h
                        mm(nc, pv[:], vaug[:, t, :], et[:, h * 512:(h + 1) * 512],
                           start=(n == 0 and h == 0),
                           stop=(n == len(iis) - 1 and h == 1))
                if kc == 0:
                    nc.vector.tensor_copy(acc[j][:], pv[:])
                else:
                    nc.vector.tensor_add(acc[j][:], acc[j][:], pv[:])
                if kc == j // 2:  # last k-chunk for this q-chunk: emit output
                    nc.sync.dma_start(
                        out=pvt[:, j * 512:(j + 1) * 512], in_=acc[j][:])

            # Loads ordered so exp work unlocks steadily: k0/v0 first, then
            # q7..q0 (heavy chunks early) with k1..k3 interleaved. An
            # attn_block(j, kc) may only be emitted after BOTH q_stage(j) and
            # kv_stage(kc) (program-order read-after-write).
            schedule = ["k0", "q7", "q6", "k1", "q5", "q4", "k2",
                        "q3", "q2", "q1", "k3", "q0"]
            done_q, done_k = set(), set()
            for item in schedule:
                if item[0] == "k":
                    kc = int(item[1])
                    kv_stage(kc)
                    done_k.add(kc)
                    for j in sorted(done_q, reverse=True):
                        attn_block(j, kc)
                else:
                    j = int(item[1])
                    q_stage(j)
                    done_q.add(j)
                    for kc in sorted(done_k):
                        attn_block(j, kc)
    nc.compile()
    return nc


def _prep_w(w, scale=1.0):
    # [512, 64] -> [128, 4, 64]: (p, dt, e) holds W[dt*128 + p, e] so the
    # lhsT slice [:, dt, :] matches X^T d-block dt.
    return np.ascontiguousarray(
        (w * scale).reshape(4, 128, D_OUT).transpose(1, 0, 2).astype(np.float32))


def kernel(inputs_for_keys, inputs_for_values, inputs_for_queries, WK, WV, WQ):
    xk_f = np.asarray(inputs_for_keys, np.float32)
    xv_f = np.asarray(inputs_for_values, np.float32)
    xq_f = np.asarray(inputs_for_queries, np.float32)
    wkp = _prep_w(np.asarray(WK, np.float32))
    wvp = _prep_w(np.asarray(WV, np.float32))
    wqp = _prep_w(np.asarray(WQ, np.float32), scale=1.0 / np.sqrt(np.float32(S)))

    if "nc" not in _CACHE:
        _CACHE["nc"] = build_nc()
    nc = _CACHE["nc"]

    # cmask2[k, 0:512]   : pair member h=0 -> 1 if q >= k + 128*hk
    # cmask2[k, 512:1024]: pair member h=1 -> 1 if q >= k + 256 + 128*hk
    kk = np.arange(128)[:, None]
    qq = np.arange(512)[None, :]
    cms = []
    for hk in range(2):
        m0 = (qq >= kk + 128 * hk).astype(np.float32)
        m1 = (qq >= kk + 256 + 128 * hk).astype(np.float32)
        cms.append(np.ascontiguousarray(np.concatenate([m0, m1], axis=1)))

    ones16 = np.ones((128, N_KT), np.float32)
    in_maps = []
    xqt_b = [np.ascontiguousarray(xq_f[b].T) for b in range(B)]
    for c in range(N_CORES):
        b, hk = c // 2, c % 2
        xk_g = xk_f[b].reshape(S // 128, 128, D_IN)[hk::2].reshape(SK, D_IN)
        xv_g = xv_f[b].reshape(S // 128, 128, D_IN)[hk::2].reshape(SK, D_IN)
        in_maps.append({
            "xqt": xqt_b[b],
            "xkt": np.ascontiguousarray(xk_g.T),
            "xvt": np.ascontiguousarray(xv_g.T),
            "wq": wqp, "wk": wkp, "wv": wvp,
            "cmask2": cms[hk],
            "ones16": ones16,
        })

    _CACHE["in_maps"] = in_maps
    res = run_bass_kernel_spmd(nc, in_maps, core_ids=list(range(N_CORES)))
    out = np.empty((B, S, D_OUT), np.float32)
    for b in range(B):
        p = res.results[2 * b]["pvt"] + res.results[2 * b + 1]["pvt"]
        out[b] = (p[0:D_OUT, :] / p[D_OUT:D_OUT + 1, :]).T
    return out


# revision 21
# speedup vs baseline: 216.6357x; 216.6357x over previous
"""Causal attention head kernel for Trainium2, 8 NeuronCores.

Problem: B=4, S=4096, D_IN=512, D_OUT=64, f32, causal, scale=1/sqrt(S).

Sharding: core c -> (batch b = c//2, k-shard hk = c%2). Each core handles ALL
queries of its batch but only the k-tiles (of 128 rows) with tile_index % 2 ==
hk, producing partial (numerator | denominator) sums; the host combines the
two k-shards. The instruction stream is identical across cores (SPMD):
causality differences between the two k-shards live in a small per-core mask
input (cmask2) and in the per-core gather of X_k/X_v rows. The host
pre-transposes X (layout prep) so the device loads d-on-partition tiles with
large contiguous DMAs.

Per-core device pipeline (f32 datapath, float32r matmuls: 1 cyc/row at N>=256):
  1. X^T tiles [128, 4(d-block), cols], one 1 MiB DMA per 512-col chunk,
     software-pipelined k-chunk/q-chunk schedule (k0 q7 q6 k1 q5 q4 ...).
  2. Projections: QT[64,4096], KT[64,2048] (head dim on partitions) and
     V_aug[128k, 65] = V | ones (ones column -> softmax denominator via the
     same PV matmul); V hi/lo split for k-tiles 0,1 cancels fp32r rounding
     where few-key rows can't average it out.
  3. Attention iterates k-chunk OUTER, q-chunk inner so exp work unlocks as
     soon as each k-chunk lands: S^T pair = KT_tile.T @ QT_chunk into a
     [128,1024] PSUM pair, one Exp activation per pair (PSUM -> SBUF f32r; no
     max subtraction -- scores are O(1) by construction: Q pre-scaled by
     1/sqrt(S)), diagonal-pair causal mask via elementwise mul, PV
     accumulation psum[65,512] per (q-chunk, k-chunk), DVE-accumulated into
     SBUF across k-chunks.
  4. Output PVT [65,4096] f32 per core; host: out[b] =
     ((PVT[2b] + PVT[2b+1])[0:64] / [64]).T
"""

import numpy as np

import concourse.bass as bass
import concourse.bacc as bacc
import concourse.mybir as mybir
from concourse import tile
from concourse.bass_utils import run_bass_kernel_spmd

F32 = mybir.dt.float32
F32R = mybir.dt.float32r

B, S, D_IN, D_OUT = 4, 4096, 512, 64
SK = S // 2          # per-core k rows (interleaved 128-tiles)
N_KT = SK // 128     # 16 local k-tiles
N_QC = S // 512      # 8 q-chunks of 512
N_CORES = 8

_CACHE = {}


def mm(nc, out, lhsT, rhs, start, stop):
    nc.tensor.matmul(out, lhsT, rhs, start=start, stop=stop)


def build_nc():
    nc = bacc.Bacc(trn_type="TRN2", target_bir_lowering=False, debug=False)

    xqt_d = nc.dram_tensor("xqt", [D_IN, S], F32R, kind="ExternalInput").ap()
    xkt_d = nc.dram_tensor("xkt", [D_IN, SK], F32R, kind="ExternalInput").ap()
    xvt_d = nc.dram_tensor("xvt", [D_IN, SK], F32R, kind="ExternalInput").ap()
    wq = nc.dram_tensor("wq", [128, 4, D_OUT], F32R, kind="ExternalInput").ap()
    wk = nc.dram_tensor("wk", [128, 4, D_OUT], F32R, kind="ExternalInput").ap()
    wv = nc.dram_tensor("wv", [128, 4, D_OUT], F32R, kind="ExternalInput").ap()
    cm = nc.dram_tensor("cmask2", [128, 1024], F32R, kind="ExternalInput").ap()
    ones_d = nc.dram_tensor("ones16", [128, N_KT + 2], F32R, kind="ExternalInput").ap()
    pvt = nc.dram_tensor("pvt", [D_OUT + 1, S], F32, kind="ExternalOutput").ap()

    with tile.TileContext(nc) as tc:
        with (
            tc.tile_pool(name="persist", bufs=1) as pp,
            tc.tile_pool(name="et", bufs=3) as etp,
            tc.tile_pool(name="ostage", bufs=2) as osp,
            tc.tile_pool(name="ps_s", bufs=2, space="PSUM") as ps_s,
            tc.tile_pool(name="ps_pv", bufs=2, space="PSUM") as ps_pv,
            tc.tile_pool(name="ps_pr", bufs=2, space="PSUM") as ps_pr,
        ):
            # ---- persistent SBUF tiles ----
            # [128, 4(d-block), cols]: one 1 MiB DMA per 512-col chunk
            xqT = pp.tile([128, 4, S], F32R, tag="xqT", name="xqT")
            xkT = pp.tile([128, 4, SK], F32R, tag="xkT", name="xkT")
            xvT = pp.tile([128, 4, SK], F32R, tag="xvT", name="xvT")
            qt = pp.tile([64, S], F32R, tag="qt", name="qt")
            kt = pp.tile([64, SK], F32R, tag="kt", name="kt")
            vaug = pp.tile([128, N_KT, D_OUT + 1], F32R, tag="vaug", name="vaug")
            vaug_lo = pp.tile([128, 2, D_OUT + 1], F32R, tag="vaug_lo",
                              name="vaug_lo")
            cmask = pp.tile([128, 1024], F32R, tag="cmask", name="cmask")
            w_sb = {}
            for nm, src in (("wq", wq), ("wk", wk), ("wv", wv)):
                w_sb[nm] = pp.tile([128, 4, D_OUT], F32R, tag=nm, name=f"{nm}_sb")
                nc.sync.dma_start(out=w_sb[nm][:], in_=src[:])
            nc.sync.dma_start(out=cmask[:], in_=cm[:])
            # ones column for the softmax denominator
            nc.sync.dma_start(out=vaug[:, :, D_OUT], in_=ones_d[:, 0:N_KT])
            nc.sync.dma_start(out=vaug_lo[:, :, D_OUT], in_=ones_d[:, N_KT:])

            # ---- software-pipelined loads -> projections -> attention ----
            # Column-chunked loads (512 cols = 256 KiB per DMA) so q-chunk 0's
            # dependencies clear early and later loads overlap attention.
            def load_cols(dst_tile, src_ap, c, ncols):
                sl = slice(c * 512, (c + 1) * 512)
                src = src_ap.rearrange("(db p) c -> p db c", p=128)
                nc.sync.dma_start(out=dst_tile[:, :, sl], in_=src[:, :, sl])

            def proj_chunk(dst, xT, w, c, nm):
                sl = slice(c * 512, (c + 1) * 512)
                ps = ps_pr.tile([128, 512], F32, tag="ps_pr", name=f"pp_{nm}{c}")
                for dt in range(4):
                    mm(nc, ps[0:64, :], w[:, dt, :], xT[:, dt, sl],
                       start=(dt == 0), stop=(dt == 3))
                nc.vector.tensor_copy(dst[:, sl], ps[0:64, :])

            def v_chunk(c):
                # V natural for s-tiles 4c..4c+3: [128s,64] per tile
                for t in range(4 * c, 4 * c + 4):
                    ps = ps_pr.tile([128, 512], F32, tag="ps_pr", name=f"pv_{t}")
                    for dt in range(4):
                        mm(nc, ps[:, 0:D_OUT],
                           xvT[:, dt, t * 128:(t + 1) * 128],
                           w_sb["wv"][:, dt, :],
                           start=(dt == 0), stop=(dt == 3))
                    nc.vector.tensor_copy(vaug[:, t, 0:D_OUT], ps[:, 0:D_OUT])
                    if t < 2:  # hi/lo split: lo = exact - rounded(hi)
                        nc.vector.tensor_sub(
                            vaug_lo[:, t, 0:D_OUT], ps[:, 0:D_OUT],
                            vaug[:, t, 0:D_OUT])

            # SBUF accumulators for the k-chunk-outer PV partial sums
            acc = [pp.tile([65, 512], F32, tag=f"acc{j}", name=f"acc{j}")
                   for j in range(N_QC)]

            def kv_stage(kc):
                load_cols(xkT, xkt_d, kc, SK)
                load_cols(xvT, xvt_d, kc, SK)
                proj_chunk(kt, xkT, w_sb["wk"], kc, "k")
                v_chunk(kc)

            def q_stage(j):
                load_cols(xqT, xqt_d, j, S)
                proj_chunk(qt, xqT, w_sb["wq"], j, "q")

            def attn_block(j, kc):
                # pairs i of q-chunk j whose k-tiles (2i, 2i+1) lie in k-chunk
                # kc; i == j is the diagonal (masked) pair.
                iis = [i for i in (2 * kc, 2 * kc + 1) if i <= j]
                if not iis:
                    return
                qs = qt[:, j * 512:(j + 1) * 512]
                pv = ps_pv.tile([65, 512], F32, tag="ps_pv", name=f"pvp{j}_{kc}")
                pv_mms = []  # (lhsT, rhs) accumulation group, flags at end
                ets = {}
                for n, i in enumerate(iis):
                    ps = ps_s.tile([128, 1024], F32, tag="ps_s", name=f"st{j}_{i}")
                    for h in range(2):
                        t = 2 * i + h
                        mm(nc, ps[:, h * 512:(h + 1) * 512],
                           kt[:, t * 128:(t + 1) * 128], qs,
                           start=True, stop=True)
                    et = etp.tile([128, 1024], F32R, tag="et", name=f"et{j}_{i}")
                    nc.scalar.activation(
                        et[:], ps[:], mybir.ActivationFunctionType.Exp)
                    if i == j:  # diagonal pair: causal mask
                        nc.vector.tensor_mul(et[:], et[:], cmask[:])
                    for h in range(2):
                        t = 2 * i + h
                        eh = et[:, h * 512:(h + 1) * 512]
                        pv_mms.append((vaug[:, t, :], eh))
                        if i == 0:
                            pv_mms.append((vaug_lo[:, t, :], eh))
                for n, (lh, rh) in enumerate(pv_mms):
                    mm(nc, pv[:], lh, rh,
                       start=(n == 0), stop=(n == len(pv_mms) - 1))
                if kc == 0:
                    nc.vector.tensor_copy(acc[j][:], pv[:])
                else:
                    nc.vector.tensor_add(acc[j][:], acc[j][:], pv[:])
                if kc == j // 2:  # last k-chunk for this q-chunk: emit output
                    nc.sync.dma_start(
                        out=pvt[:, j * 512:(j + 1) * 512], in_=acc[j][:])

            # Loads ordered so exp work unlocks steadily: k0/v0 first, then
            # q7..q0 (heavy chunks early) with k1..k3 interleaved. An
            # attn_block(j, kc) may only be emitted after BOTH q_stage(j) and
            # kv_stage(kc) (program-order read-after-write).
            schedule = ["k0", "q7", "q6", "k1", "q5", "q4", "k2",
                        "q3", "q2", "q1", "k3", "q0"]
            done_q, done_k = set(), set()
            for item in schedule:
                if item[0] == "k":
                    kc = int(item[1])
                    kv_stage(kc)
                    done_k.add(kc)
                    for j in sorted(done_q, reverse=True):
                        attn_block(j, kc)
                else:
                    j = int(item[1])
                    q_stage(j)
                    done_q.add(j)
                    for kc in sorted(done_k):
                        attn_block(j, kc)
    nc.compile()
    return nc


def _prep_w(w, scale=1.0):
    # [512, 64] -> [128, 4, 64]: (p, dt, e) holds W[dt*128 + p, e] so the
    # lhsT slice [:, dt, :] matches X^T d-block dt.
    return np.ascontiguousarray(
        (w * scale).reshape(4, 128, D_OUT).transpose(1, 0, 2).astype(np.float32))


def kernel(inputs_for_keys, inputs_for_values, inputs_for_queries, WK, WV, WQ):
    xk_f = np.asarray(inputs_for_keys, np.float32)
    xv_f = np.asarray(inputs_for_values, np.float32)
    xq_f = np.asarray(inputs_for_queries, np.float32)
    wkp = _prep_w(np.asarray(WK, np.float32))
    wvp = _prep_w(np.asarray(WV, np.float32))
    wqp = _prep_w(np.asarray(WQ, np.float32), scale=1.0 / np.sqrt(np.float32(S)))

    if "nc" not in _CACHE:
        _CACHE["nc"] = build_nc()
    nc = _CACHE["nc"]

    # cmask2[k, 0:512]   : pair member h=0 -> 1 if q >= k + 128*hk
    # cmask2[k, 512:1024]: pair member h=1 -> 1 if q >= k + 256 + 128*hk
    kk = np.arange(128)[:, None]
    qq = np.arange(512)[None, :]
    cms = []
    for hk in range(2):
        m0 = (qq >= kk + 128 * hk).astype(np.float32)
        m1 = (qq >= kk + 256 + 128 * hk).astype(np.float32)
        cms.append(np.ascontiguousarray(np.concatenate([m0, m1], axis=1)))

    ones16 = np.concatenate([np.ones((128, N_KT), np.float32),
                         np.zeros((128, 2), np.float32)], axis=1)
    in_maps = []
    xqt_b = [np.ascontiguousarray(xq_f[b].T) for b in range(B)]
    for c in range(N_CORES):
        b, hk = c // 2, c % 2
        xk_g = xk_f[b].reshape(S // 128, 128, D_IN)[hk::2].reshape(SK, D_IN)
        xv_g = xv_f[b].reshape(S // 128, 128, D_IN)[hk::2].reshape(SK, D_IN)
        in_maps.append({
            "xqt": xqt_b[b],
            "xkt": np.ascontiguousarray(xk_g.T),
            "xvt": np.ascontiguousarray(xv_g.T),
            "wq": wqp, "wk": wkp, "wv": wvp,
            "cmask2": cms[hk],
            "ones16": ones16,
        })

    _CACHE["in_maps"] = in_maps
    res = run_bass_kernel_spmd(nc, in_maps, core_ids=list(range(N_CORES)))
    out = np.empty((B, S, D_OUT), np.float32)
    for b in range(B):
        p = res.results[2 * b]["pvt"] + res.results[2 * b + 1]["pvt"]
        out[b] = (p[0:D_OUT, :] / p[D_OUT:D_OUT + 1, :]).T
    return out
